# revision 1
# baseline (speedup 1.0000x reference)
"""BinaryLinear (sign-binarized weight linear layer) on 8 Trainium2 NeuronCores.

y[b,s,o] = sum_i x[b,s,i] * (scale[o] * sign(w[o,i])) + bias[o]
  with scale[o] = mean_i |w[o,i]|

Sharding: data-parallel over the batch dim (8 batches -> 8 cores); w/bias
replicated. Per core (m = sequence rows, o = out features, k = in features):

  - x f32 -> bf16 via SWDGE cast-DMA (DRAM->DRAM), then XBAR DMA-transpose
    into a fully SBUF-resident x^T [k-part, m] (16.8 MB, read by every matmul)
  - w binarized on-chip: ACT Sign -> bf16 B rows, DVE abs-row-sum -> scale;
    B written back to DRAM and XBAR-transposed to B^T [k-part, o] tiles
  - TensorE: yT[o,m] = B^T.T @ x^T accumulated over k in PSUM (bf16 inputs,
    f32 accumulate); DVE fuses psum*scale[o]+bias[o] on PSUM eviction
  - phases ordered so weight prep & x^T build hide under the first n-column
    of matmuls; B^T tiles for the early o-rows are streamed twice to allow it

Host side only shards inputs and transposes yT shards back into y.
"""

import numpy as np

B_DIM = 8
S_DIM = 2048
IN_F = 4096
OUT_F = 4096
P = 128
N_CORES = 8
N_TILE = 512

_BUILT = None


def _build_nc(s_dim=S_DIM, in_f=IN_F, out_f=OUT_F):
    from contextlib import ExitStack

    import concourse.mybir as mybir
    import concourse.tile as tile
    from concourse import bacc
    from concourse.bass import ts

    f32 = mybir.dt.float32
    bf16 = mybir.dt.bfloat16

    NCH = s_dim // N_TILE  # n chunks (moving-dim tiles of 512)
    PO = out_f // P  # o blocks (output-partition tiles of 128)
    KT = in_f // P  # contraction subtiles of 128
    HALF = in_f // 2
    # phase A runs n=0 for the first A_N o-blocks so the x^T chunk builds
    # (cast + transpose DMAs) can finish in their shadow
    A_N = min(6, PO) if NCH > 1 else PO

    nc = bacc.Bacc(None, target_bir_lowering=False, debug=False)
    with tile.TileContext(nc) as tc:
        x_d = nc.dram_tensor("x", (s_dim, in_f), f32, kind="ExternalInput")
        w_d = nc.dram_tensor("w", (out_f, in_f), f32, kind="ExternalInput")
        b_d = nc.dram_tensor("bias", (out_f,), f32, kind="ExternalInput")
        yT_d = nc.dram_tensor("yT", (out_f, s_dim), f32, kind="ExternalOutput")

        with ExitStack() as ctx:
            dram = ctx.enter_context(tc.tile_pool(name="dram", bufs=1, space="DRAM"))
            xbf_d = dram.tile((s_dim, in_f), bf16)
            bw_d = dram.tile((out_f, in_f), bf16)
            # 3D views: innermost 128 of k becomes the SBUF partition dim
            xbf3 = xbf_d[:, :].rearrange("m (po pi) -> m po pi", pi=P)
            bw3 = bw_d[:, :].rearrange("o (po pi) -> o po pi", pi=P)
            yT3 = yT_d[:, :].rearrange("(po pi) s -> pi po s", pi=P)

            const = ctx.enter_context(tc.tile_pool(name="const", bufs=1))
            xT = const.tile([P, NCH, KT, N_TILE], bf16)  # resident x^T
            scale_sb = const.tile([P, PO], f32)
            bias_sb = const.tile([P, PO], f32)
            nc.sync.dma_start(bias_sb[:], b_d[:].rearrange("(po pi) -> pi po", pi=P))

            wpool = ctx.enter_context(tc.tile_pool(name="wpool", bufs=4))
            bpool = ctx.enter_context(tc.tile_pool(name="bpool", bufs=2))
            btpool = ctx.enter_context(tc.tile_pool(name="btpool", bufs=2))
            scpool = ctx.enter_context(tc.tile_pool(name="scpool", bufs=2))
            opool = ctx.enter_context(tc.tile_pool(name="opool", bufs=7))
            psum = ctx.enter_context(tc.tile_pool(name="psum", bufs=6, space="PSUM"))

            def cast_x_chunk(c):
                # cast 512 rows f32->bf16 (SWDGE, DRAM->DRAM); the casts
                # serialize on the gpsimd queue, so issue them all up front
                nc.gpsimd.dma_start(xbf_d[ts(c, N_TILE), :], x_d[ts(c, N_TILE), :])

            def build_x_chunk(c):
                # XBAR-transpose a cast chunk into the resident x^T, in 1 MB
                # pieces (a DMA_TRANSPOSE occupies its queue for the whole
                # transfer; pieces let other sync work interleave)
                npc = max(1, KT // 8)
                for p in range(npc):
                    nc.sync.dma_start_transpose(
                        xT[:, c, ts(p, KT // npc)],
                        xbf3[ts(c, N_TILE), ts(p, KT // npc)],
                    )

            # W prep is software-pipelined at emission time: the ACT queue is
            # strictly in-order, so a load->sign->write->transpose chain for
            # one block stalls the queue on every DMA completion. Loading
            # block m+2 and signing block m+1 while m's B^T transposes keeps
            # the queue from ever waiting.
            w_tiles = {}

            def load_w(m):
                halves = []
                for h in range(2):
                    w_sb = wpool.tile([P, HALF], f32, tag="w", name=f"w_{m}_{h}")
                    nc.scalar.dma_start(w_sb[:], w_d[ts(m, P), ts(h, HALF)])
                    halves.append(w_sb)
                w_tiles[m] = halves

            def process_w(m):
                # sign -> bf16 B rows (ACT), |w| row sums -> scale (DVE)
                b_sb = bpool.tile([P, in_f], bf16)
                sc2 = scpool.tile([P, 2], f32)
                for h in range(2):
                    w_sb = w_tiles[m][h]
                    nc.scalar.sign(b_sb[:, ts(h, HALF)], w_sb[:])
                    nc.vector.tensor_reduce(
                        sc2[:, h : h + 1],
                        w_sb[:],
                        axis=mybir.AxisListType.X,
                        op=mybir.AluOpType.add,
                        apply_absolute_value=True,
                    )
                del w_tiles[m]
                nc.vector.tensor_reduce(
                    scale_sb[:, m : m + 1],
                    sc2[:],
                    axis=mybir.AxisListType.X,
                    op=mybir.AluOpType.add,
                )
                nc.vector.tensor_scalar_mul(
                    scale_sb[:, m : m + 1], scale_sb[:, m : m + 1], 1.0 / in_f
                )
                nc.scalar.dma_start(bw_d[ts(m, P), :], b_sb[:])

            def load_bt(m, eng=None):
                # DMA_TRANSPOSE occupies its issuing queue for the whole
                # transfer; route to ACT by default, Sync when ACT is busy
                bt = btpool.tile([P, KT, P], bf16)
                (eng or nc.scalar).dma_start_transpose(bt[:], bw3[ts(m, P)])
                return bt

            def mm_block(bt, m, n):
                ps = psum.tile([P, N_TILE], f32, name="ps")
                for kt in range(KT):
                    nc.tensor.matmul(
                        ps[:],
                        bt[:, kt, :],
                        xT[:, n, kt, :],
                        start=(kt == 0),
                        stop=(kt == KT - 1),
                    )
                ob = opool.tile([P, N_TILE], f32)
                nc.vector.tensor_scalar(
                    ob[:],
                    ps[:],
                    scale_sb[:, m : m + 1],
                    bias_sb[:, m : m + 1],
                    op0=mybir.AluOpType.mult,
                    op1=mybir.AluOpType.add,
                )
                nc.sync.dma_start(yT3[:, m, ts(n, N_TILE)], ob[:])

            # x pipeline first: casts chain on gpsimd, transpose pieces queue
            # on sync and fire as their cast completes
            for c in range(NCH):
                cast_x_chunk(c)
            for c in range(NCH):
                build_x_chunk(c)

            # W-prep runs `next_proc` blocks ahead of consumption
            load_w(0)
            load_w(1)
            process_w(0)
            next_proc = 1

            def advance_prep():
                nonlocal next_proc
                if next_proc < PO:
                    if next_proc + 1 < PO:
                        load_w(next_proc + 1)
                    process_w(next_proc)
                    next_proc += 1

            # phase A: n=0 for the first A_N o-blocks while x^T chunks build
            for m in range(A_N):
                bt = load_bt(m)
                advance_prep()
                mm_block(bt, m, 0)
            # phase B: remaining n for those o-blocks (B^T tiles re-streamed)
            if NCH > 1:
                for m in range(A_N):
                    bt = load_bt(m, eng=nc.scalar if m % 2 else nc.sync)
                    advance_prep()
                    for n in range(1, NCH):
                        mm_block(bt, m, n)
            # phase C: the rest, n inner
            for m in range(A_N, PO):
                bt = load_bt(m, eng=nc.scalar if m % 2 else nc.sync)
                advance_prep()
                for n in range(NCH):
                    mm_block(bt, m, n)
    nc.finalize()
    return nc


def _get_nc():
    global _BUILT
    if _BUILT is None:
        _BUILT = _build_nc()
    return _BUILT


def kernel(x, weight, bias):
    from concourse.bass_utils import run_bass_kernel_spmd

    x = np.asarray(x, dtype=np.float32)
    weight = np.asarray(weight, dtype=np.float32)
    bias = np.asarray(bias, dtype=np.float32)
    assert x.shape == (B_DIM, S_DIM, IN_F), x.shape

    nc = _get_nc()
    in_maps = [
        {"x": np.ascontiguousarray(x[b]), "w": weight, "bias": bias}
        for b in range(N_CORES)
    ]
    res = run_bass_kernel_spmd(nc, in_maps, core_ids=list(range(N_CORES)))
    out = np.empty((B_DIM, S_DIM, OUT_F), dtype=np.float32)
    for b in range(N_CORES):
        out[b] = res.results[b]["yT"].T
    return out

